# revision 29
# baseline (speedup 1.0000x reference)
"""Articulation (LBS) kernel for 8 TRN2 NeuronCores.

Math: V[b,v,m] = sum_j w[v,j] * (A[b,j,m,:] . vh[v,:]), vh = [v_template, 1].
Fused as a single matmul with K = J*4 = 560 (padded 640):
  out[(b,m), v] = sum_k A2[k, (b,m)] * W2[k, v]
where A2[(j,n),(b,m)] = A[b,j,m,n] (stationary, tiny, replicated) and
W2[(j,n), v] = w[v,j]*vh[v,n] (moving, large, sharded over v across cores).

Host does the tiny serial work (Rodrigues, FK tree walk, keypoints); the
device does the O(B*VN*J) blend. VN=14000 is split 1750/core across 8 cores.
"""

import sys

sys.path.insert(0, "/opt/trn_rl_repo")

import numpy as np

import concourse.bacc as bacc
import concourse.bass as bass
from concourse import mybir
from concourse.bass_utils import run_bass_kernel_spmd
from concourse.tile import TileContext

B, J, VN = 256, 140, 14000
NCORES = 8
VSH = VN // NCORES          # 1750 vertices per core
KC = 5                      # k chunks of 128 (J*4 = 560 padded to 640)
BM = B * 3                  # 768 output rows (b, m)
NG = BM // 128              # 6 stationary groups of 128 output columns
_SIZES = [512, 512, 512, 214]  # big chunks: each compute window covers next DMA
CHUNKS = []
_o = 0
for _s in _SIZES:
    CHUNKS.append((_o, _s))
    _o += _s
assert _o == VSH

_cache = {}

USE_FP8 = True
KCD = 5                      # fp8: 2 DoubleRow pairs (K 0..511) + 1 plain MM (48-row tail)
VSHP = 1760                  # per-core v columns padded to %16 for DoubleRow APs
CHUNKS8 = [(0, 512), (512, 512), (1024, 512), (1536, 224)]


def _build_program_fp8():
    """fp8 e4m3 + DoubleRow: 2 K-rows per PE cell -> 3 passes instead of 5.
    Casts split 2:1 between DVE and ACT (both may read PSUM, different banks)."""
    if "nc8" in _cache:
        return _cache["nc8"]
    nc = bacc.Bacc(None)
    f8 = mybir.dt.float8e4
    f16 = mybir.dt.float16
    f32 = mybir.dt.float32
    w2 = nc.declare_dram_parameter("w2", [128, KCD, VSHP], f8, isOutput=False)
    a2 = nc.declare_dram_parameter("a2", [128, KCD, BM], f8, isOutput=False)
    out = nc.declare_dram_parameter("out", [NG, 128, VSHP], f16, isOutput=True)
    out_r = out.rearrange("g p v -> p g v")

    with TileContext(nc) as tc:
        with (
            tc.tile_pool(name="a2p", bufs=1) as a2p,
            tc.tile_pool(name="w2p", bufs=3) as w2p,
            tc.tile_pool(name="op", bufs=2) as op,
            tc.tile_pool(name="psp", bufs=6, space="PSUM") as psp,
        ):
            a2t0 = a2p.tile([128, 2, BM], f8, tag="a2_0")
            a2t1 = a2p.tile([128, 2, BM], f8, tag="a2_1")
            a2ts = [a2t0, a2t1]
            a2tail = a2p.tile([128, BM], f8, tag="a2_t")
            first = True
            gi = 0
            for c0, cs in CHUNKS8:
                w2t = w2p.tile([128, KCD, cs], f8, tag="w2")
                if first:
                    nc.sync.dma_start(a2ts[0][:], a2[:, 0:2, :])
                nc.sync.dma_start(w2t[:], w2[:, :, c0 : c0 + cs])
                if first:
                    nc.sync.dma_start(a2ts[1][:], a2[:, 2:4, :])
                    nc.sync.dma_start(a2tail[:], a2[:, 4, :])
                    first = False
                ot = op.tile([128, NG, cs], f16, tag="ot")
                for g in range(NG):
                    ps = psp.tile([128, cs], f32, tag="ps")
                    for p in range(2):
                        nc.tensor.matmul(
                            ps[:],
                            lhsT=a2ts[p][:, :, g * 128 : (g + 1) * 128],
                            rhs=w2t[:, 2 * p : 2 * p + 2, :],
                            start=(p == 0),
                            stop=False,
                            perf_mode=mybir.MatmulPerfMode.DoubleRow,
                        )
                    nc.tensor.matmul(
                        ps[:],
                        lhsT=a2tail[:, g * 128 : (g + 1) * 128],
                        rhs=w2t[:, 4, :],
                        start=False,
                        stop=True,
                    )
                    if gi % 3 == 2:
                        nc.scalar.copy(ot[:, g, :], ps[:])
                    else:
                        nc.vector.tensor_copy(ot[:, g, :], ps[:])
                    gi += 1
                nc.sync.dma_start(out_r[:, :, c0 : c0 + cs], ot[:])
    nc.finalize()
    _cache["nc8"] = nc
    return nc


def _build_program_raw():
    """Hand-scheduled raw-Bass version: explicit engine streams + 4 semaphores.

    Avoids Tile/Bacc's event-semaphore system whose prologue/teardown
    (all-engine barriers, ~253 per-sem resets) costs ~15us of fixed time.
    """
    if "nc_raw" in _cache:
        return _cache["nc_raw"]
    nc = bass.Bass()
    f16 = mybir.dt.float16
    f32 = mybir.dt.float32
    w2 = nc.declare_dram_parameter("w2", [128, KC, VSH], f16, isOutput=False)
    a2 = nc.declare_dram_parameter("a2", [128, KC, BM], f16, isOutput=False)
    out = nc.declare_dram_parameter("out", [NG, 128, VSH], f16, isOutput=True)
    out_r = out.rearrange("g p v -> p g v")

    NBANK = 8
    groups = [(ci, g) for ci in range(len(CHUNKS)) for g in range(NG)]

    with (
        nc.sbuf_tensor([128, KC, BM], f16) as a2_sb,
        nc.sbuf_tensor([128, KC, VSH], f16) as w2_sb,
        nc.sbuf_tensor([128, NG, VSH], f16) as ot_sb,
        nc.psum_tensor([128, NBANK, 512], f32) as ps,
        nc.semaphore("s_a2_0") as s_a2_0,
        nc.semaphore("s_a2_1") as s_a2_1,
        nc.semaphore("s_a2_2") as s_a2_2,
        nc.semaphore("s_a2_3") as s_a2_3,
        nc.semaphore("s_a2_4") as s_a2_4,
        nc.semaphore("s_w2_0") as s_w2_0,
        nc.semaphore("s_w2_1") as s_w2_1,
        nc.semaphore("s_w2_2") as s_w2_2,
        nc.semaphore("s_w2_3") as s_w2_3,
        nc.semaphore("s_w2_4") as s_w2_4,
        nc.semaphore("s_mm") as s_mm,
        nc.semaphore("s_cast") as s_cast,
        nc.semaphore("s_out") as s_out,
        nc.Block() as block,
    ):
        s_a2 = [s_a2_0, s_a2_1, s_a2_2, s_a2_3, s_a2_4]
        s_w2 = [s_w2_0, s_w2_1, s_w2_2, s_w2_3, s_w2_4]

        @block.sync
        def _(sync):
            # input DMAs in consumption order; one semaphore per DMA so a
            # full-value wait is race-free (HWDGE completions can skew).
            c0, cs = CHUNKS[0]
            sync.dma_start(a2_sb[:, 0, :], a2[:, 0, :]).then_inc(s_a2[0], 16)
            sync.dma_start(
                w2_sb[:, :, c0 : c0 + cs], w2[:, :, c0 : c0 + cs]
            ).then_inc(s_w2[0], 16)
            for kc in range(1, KC):
                sync.dma_start(a2_sb[:, kc, :], a2[:, kc, :]).then_inc(s_a2[kc], 16)
            for ci in range(1, len(CHUNKS)):
                c0, cs = CHUNKS[ci]
                sync.dma_start(
                    w2_sb[:, :, c0 : c0 + cs], w2[:, :, c0 : c0 + cs]
                ).then_inc(s_w2[ci], 16)
            # output DMAs, one per (chunk, group), as casts complete
            for i, (ci, g) in enumerate(groups):
                c0, cs = CHUNKS[ci]
                sync.wait_ge(s_cast, i + 1)
                sync.dma_start(
                    out_r[:, g : g + 1, c0 : c0 + cs], ot_sb[:, g : g + 1, c0 : c0 + cs]
                ).then_inc(s_out, 16)
            sync.wait_ge(s_out, 16 * len(groups))

        @block.tensor
        def _(tensor):
            for i, (ci, g) in enumerate(groups):
                c0, cs = CHUNKS[ci]
                bank = i % NBANK
                if i >= NBANK:
                    tensor.wait_ge(s_cast, i - NBANK + 1)
                if g == 0:
                    tensor.wait_ge(s_w2[ci], 16)
                for kc in range(KC):
                    if ci == 0 and g == 0:
                        tensor.wait_ge(s_a2[kc], 16)
                    mm = nc.tensor.matmul(
                        ps[:, bank, :cs],
                        lhsT=a2_sb[:, kc, g * 128 : (g + 1) * 128],
                        rhs=w2_sb[:, kc, c0 : c0 + cs],
                        start=(kc == 0),
                        stop=(kc == KC - 1),
                    )
                    if kc == KC - 1:
                        mm.then_inc(s_mm, 1)

        @block.vector
        def _(vector):
            for i, (ci, g) in enumerate(groups):
                c0, cs = CHUNKS[ci]
                bank = i % NBANK
                vector.wait_ge(s_mm, i + 1)
                nc.vector.tensor_copy(
                    ot_sb[:, g, c0 : c0 + cs], ps[:, bank, :cs]
                ).then_inc(s_cast, 1)

    nc.finalize()
    _cache["nc_raw"] = nc
    return nc


def _build_program():
    if "nc" in _cache:
        return _cache["nc"]
    nc = bacc.Bacc(None)
    f16 = mybir.dt.float16
    f32 = mybir.dt.float32
    w2 = nc.declare_dram_parameter("w2", [128, KC, VSH], f16, isOutput=False)
    a2 = nc.declare_dram_parameter("a2", [128, KC, BM], f16, isOutput=False)
    out = nc.declare_dram_parameter("out", [NG, 128, VSH], f16, isOutput=True)
    out_r = out.rearrange("g p v -> p g v")

    with TileContext(nc) as tc:
        with (
            tc.tile_pool(name="a2p", bufs=1) as a2p,
            tc.tile_pool(name="w2p", bufs=3) as w2p,
            tc.tile_pool(name="op", bufs=2) as op,
            tc.tile_pool(name="psp", bufs=6, space="PSUM") as psp,
        ):
            # per-kc A2 tiles so the first matmul only waits on 1/5 of A2
            a2ts = []
            for kc in range(KC):
                t = a2p.tile([128, BM], f16, tag=f"a2_{kc}")
                a2ts.append(t)
            first = True
            for c0, cs in CHUNKS:
                w2t = w2p.tile([128, KC, cs], f16, tag="w2")
                if first:
                    nc.sync.dma_start(a2ts[0][:], a2[:, 0, :])
                nc.sync.dma_start(w2t[:], w2[:, :, c0 : c0 + cs])
                if first:
                    for kc in range(1, KC):
                        nc.sync.dma_start(a2ts[kc][:], a2[:, kc, :])
                    first = False
                ot = op.tile([128, NG, cs], f16, tag="ot")
                for g in range(NG):
                    ps = psp.tile([128, cs], f32, tag="ps")
                    for kc in range(KC):
                        nc.tensor.matmul(
                            ps[:],
                            lhsT=a2ts[kc][:, g * 128 : (g + 1) * 128],
                            rhs=w2t[:, kc, :],
                            start=(kc == 0),
                            stop=(kc == KC - 1),
                        )
                    nc.vector.tensor_copy(ot[:, g, :], ps[:])
                nc.sync.dma_start(out_r[:, :, c0 : c0 + cs], ot[:])
    nc.finalize()
    _cache["nc"] = nc
    return nc


def _rodrigues(r):
    theta = np.linalg.norm(r, axis=-1, keepdims=True) + 1e-8
    rh = r / theta
    c = np.cos(theta)[..., None]
    s = np.sin(theta)[..., None]
    x, y, z = rh[..., 0], rh[..., 1], rh[..., 2]
    zero = np.zeros_like(x)
    K = np.stack([zero, -z, y, z, zero, -x, -y, x, zero], axis=-1).reshape(
        r.shape[:-1] + (3, 3)
    )
    eye = np.eye(3, dtype=r.dtype)
    outer = rh[..., :, None] * rh[..., None, :]
    return c * eye + (1.0 - c) * outer + s * K


def kernel(
    pose_rot_vec,
    pose_trans,
    v_template,
    weights,
    tpose_joints,
    parents,
    kp_v_ids,
    kp_j_ids,
    kp_is_joint,
):
    pose_rot_vec = np.asarray(pose_rot_vec, dtype=np.float32)
    pose_trans = np.asarray(pose_trans, dtype=np.float32)
    v_template = np.asarray(v_template, dtype=np.float32)
    weights = np.asarray(weights, dtype=np.float32)
    tpose_joints = np.asarray(tpose_joints, dtype=np.float32)
    parents_np = np.asarray(parents)
    kp_v_ids = np.asarray(kp_v_ids)
    kp_j_ids = np.asarray(kp_j_ids)
    kp_is_joint = np.asarray(kp_is_joint)

    b = pose_rot_vec.shape[0]
    # ---- host: Rodrigues + FK along the tree (tiny serial compute) ----
    R = _rodrigues(pose_rot_vec)  # [B,J,3,3]
    S = np.zeros((b, J, 4, 4), dtype=np.float32)
    S[:, :, :3, :3] = R
    S[:, :, :3, 3] = pose_trans
    S[:, :, 3, 3] = 1.0
    G = np.empty_like(S)
    G[:, 0] = S[:, 0]
    for j in range(1, J):
        G[:, j] = G[:, int(parents_np[j])] @ S[:, j]
    J_final = np.ascontiguousarray(G[:, :, :3, 3])  # [B,J,3]
    A = G.copy()
    A[:, :, :3, 3] -= np.einsum("bjmn,jn->bjm", G[:, :, :3, :3], tpose_joints)

    # ---- host: pack device operands ----
    # A2[(j,n),(b,m)] = A[b,j,m,n]; W2[(j,n), v] = w[v,j]*vh[v,n]
    A2 = A[:, :, :3, :].transpose(1, 3, 0, 2).reshape(J * 4, b * 3)
    vh = np.concatenate([v_template, np.ones((VN, 1), np.float32)], axis=1)  # [VN,4]
    W2 = (weights[:, :, None] * vh[:, None, :]).reshape(VN, J * 4)

    kdim = (KCD if USE_FP8 else KC) * 128
    dt_in = mybir.dt.np(mybir.dt.float8e4) if USE_FP8 else np.float16
    kc = KCD if USE_FP8 else KC
    vshp = VSHP if USE_FP8 else VSH

    A2p = np.zeros((kdim, BM), dtype=dt_in)
    A2p[: J * 4] = A2.astype(dt_in)
    A2h = np.ascontiguousarray(A2p.reshape(kc, 128, BM).transpose(1, 0, 2))

    W2p = np.zeros((kdim, VN), dtype=dt_in)
    W2p[: J * 4] = W2.astype(dt_in).T
    W2h = np.ascontiguousarray(W2p.reshape(kc, 128, VN).transpose(1, 0, 2))

    nc = _build_program_fp8() if USE_FP8 else _build_program()
    in_maps = []
    for i in range(NCORES):
        w2c = np.zeros((128, kc, vshp), dtype=dt_in)
        w2c[:, :, :VSH] = W2h[:, :, i * VSH : (i + 1) * VSH]
        in_maps.append({"w2": w2c, "a2": A2h})
    res = run_bass_kernel_spmd(nc, in_maps, core_ids=list(range(NCORES)))
    kernel._last = res

    outs = [
        res.results[i]["out"].reshape(BM, vshp)[:, :VSH] for i in range(NCORES)
    ]  # each [BM, VSH] f16
    Vt = np.concatenate(outs, axis=1)  # [768, 14000]
    V_final = np.ascontiguousarray(
        Vt.reshape(b, 3, VN).transpose(0, 2, 1).astype(np.float32)
    )

    # ---- host: keypoints (tiny gathers) ----
    kp_v = V_final[:, kp_v_ids].mean(axis=2)  # [B,22,3]
    kp_j = J_final[:, kp_j_ids].mean(axis=2)  # [B,22,3]
    keypoints = np.where(kp_is_joint[None, :, None], kp_j, kp_v).astype(np.float32)
    return V_final, J_final, keypoints


# revision 31
# speedup vs baseline: 1.0700x; 1.0700x over previous
"""Articulation (LBS) kernel for 8 TRN2 NeuronCores.

Math: V[b,v,m] = sum_j w[v,j] * (A[b,j,m,:] . vh[v,:]), vh = [v_template, 1].
Fused as a single matmul with K = J*4 = 560 (padded 640):
  out[(b,m), v] = sum_k A2[k, (b,m)] * W2[k, v]
where A2[(j,n),(b,m)] = A[b,j,m,n] (stationary, tiny, replicated) and
W2[(j,n), v] = w[v,j]*vh[v,n] (moving, large, sharded over v across cores).

Host does the tiny serial work (Rodrigues, FK tree walk, keypoints); the
device does the O(B*VN*J) blend. VN=14000 is split 1750/core across 8 cores.
"""

import sys

sys.path.insert(0, "/opt/trn_rl_repo")

import numpy as np

import concourse.bacc as bacc
import concourse.bass as bass
from concourse import mybir
from concourse.bass_utils import run_bass_kernel_spmd
from concourse.tile import TileContext

B, J, VN = 256, 140, 14000
NCORES = 8
VSH = VN // NCORES          # 1750 vertices per core
KC = 5                      # k chunks of 128 (J*4 = 560 padded to 640)
BM = B * 3                  # 768 output rows (b, m)
NG = BM // 128              # 6 stationary groups of 128 output columns
_SIZES = [512, 512, 512, 214]  # big chunks: each compute window covers next DMA
CHUNKS = []
_o = 0
for _s in _SIZES:
    CHUNKS.append((_o, _s))
    _o += _s
assert _o == VSH

_cache = {}

USE_FP8 = True
KCD = 5                      # fp8: 2 DoubleRow pairs (K 0..511) + 1 plain MM (48-row tail)
VSHP = 1760                  # per-core v columns padded to %16 for DoubleRow APs
CHUNKS8 = [(0, 512), (512, 512), (1024, 512), (1536, 224)]


def _build_program_fp8():
    """fp8 e4m3 + DoubleRow: 2 K-rows per PE cell -> 3 passes instead of 5.
    Casts split 2:1 between DVE and ACT (both may read PSUM, different banks)."""
    if "nc8" in _cache:
        return _cache["nc8"]
    nc = bacc.Bacc(None)
    f8 = mybir.dt.float8e4
    f16 = mybir.dt.float16
    f32 = mybir.dt.float32
    w2 = nc.declare_dram_parameter("w2", [128, KCD, VSHP], f8, isOutput=False)
    a2 = nc.declare_dram_parameter("a2", [128, KCD, BM], f8, isOutput=False)
    out = nc.declare_dram_parameter("out", [NG, 128, VSHP], f16, isOutput=True)
    out_r = out.rearrange("g p v -> p g v")

    with TileContext(nc) as tc:
        with (
            tc.tile_pool(name="a2p", bufs=1) as a2p,
            tc.tile_pool(name="w2p", bufs=3) as w2p,
            tc.tile_pool(name="op", bufs=2) as op,
            tc.tile_pool(name="psp", bufs=6, space="PSUM") as psp,
            tc.tile_pool(name="wpsp", bufs=1, space="PSUM") as wpsp,
        ):
            a2t0 = a2p.tile([128, 2, BM], f8, tag="a2_0")
            a2t1 = a2p.tile([128, 2, BM], f8, tag="a2_1")
            a2ts = [a2t0, a2t1]
            a2tail = a2p.tile([128, BM], f8, tag="a2_t")
            # HAM warmup: keep PE busy during the input-DMA lead so the real
            # stream starts at 2.4 GHz.
            wrm = a2p.tile([128, 512], f16, tag="wrm")
            nc.gpsimd.memset(wrm[:], 0.0)
            wps = wpsp.tile([128, 512], f32, tag="wps")
            for _ in range(10):
                nc.tensor.matmul(
                    wps[:], lhsT=wrm[:, :128], rhs=wrm[:], start=True, stop=True
                )
            first = True
            gi = 0
            for c0, cs in CHUNKS8:
                w2t = w2p.tile([128, KCD, cs], f8, tag="w2")
                if first:
                    nc.sync.dma_start(a2ts[0][:], a2[:, 0:2, :])
                nc.sync.dma_start(w2t[:], w2[:, :, c0 : c0 + cs])
                if first:
                    nc.sync.dma_start(a2ts[1][:], a2[:, 2:4, :])
                    nc.sync.dma_start(a2tail[:], a2[:, 4, :])
                    first = False
                ot = op.tile([128, NG, cs], f16, tag="ot")
                for g in range(NG):
                    ps = psp.tile([128, cs], f32, tag="ps")
                    for p in range(2):
                        nc.tensor.matmul(
                            ps[:],
                            lhsT=a2ts[p][:, :, g * 128 : (g + 1) * 128],
                            rhs=w2t[:, 2 * p : 2 * p + 2, :],
                            start=(p == 0),
                            stop=False,
                            perf_mode=mybir.MatmulPerfMode.DoubleRow,
                        )
                    nc.tensor.matmul(
                        ps[:],
                        lhsT=a2tail[:, g * 128 : (g + 1) * 128],
                        rhs=w2t[:, 4, :],
                        start=False,
                        stop=True,
                    )
                    if gi % 3 == 2:
                        nc.scalar.copy(ot[:, g, :], ps[:])
                    else:
                        nc.vector.tensor_copy(ot[:, g, :], ps[:])
                    gi += 1
                nc.sync.dma_start(out_r[:, :, c0 : c0 + cs], ot[:])
    nc.finalize()
    _cache["nc8"] = nc
    return nc


def _build_program_raw():
    """Hand-scheduled raw-Bass version: explicit engine streams + 4 semaphores.

    Avoids Tile/Bacc's event-semaphore system whose prologue/teardown
    (all-engine barriers, ~253 per-sem resets) costs ~15us of fixed time.
    """
    if "nc_raw" in _cache:
        return _cache["nc_raw"]
    nc = bass.Bass()
    f16 = mybir.dt.float16
    f32 = mybir.dt.float32
    w2 = nc.declare_dram_parameter("w2", [128, KC, VSH], f16, isOutput=False)
    a2 = nc.declare_dram_parameter("a2", [128, KC, BM], f16, isOutput=False)
    out = nc.declare_dram_parameter("out", [NG, 128, VSH], f16, isOutput=True)
    out_r = out.rearrange("g p v -> p g v")

    NBANK = 8
    groups = [(ci, g) for ci in range(len(CHUNKS)) for g in range(NG)]

    with (
        nc.sbuf_tensor([128, KC, BM], f16) as a2_sb,
        nc.sbuf_tensor([128, KC, VSH], f16) as w2_sb,
        nc.sbuf_tensor([128, NG, VSH], f16) as ot_sb,
        nc.psum_tensor([128, NBANK, 512], f32) as ps,
        nc.semaphore("s_a2_0") as s_a2_0,
        nc.semaphore("s_a2_1") as s_a2_1,
        nc.semaphore("s_a2_2") as s_a2_2,
        nc.semaphore("s_a2_3") as s_a2_3,
        nc.semaphore("s_a2_4") as s_a2_4,
        nc.semaphore("s_w2_0") as s_w2_0,
        nc.semaphore("s_w2_1") as s_w2_1,
        nc.semaphore("s_w2_2") as s_w2_2,
        nc.semaphore("s_w2_3") as s_w2_3,
        nc.semaphore("s_w2_4") as s_w2_4,
        nc.semaphore("s_mm") as s_mm,
        nc.semaphore("s_cast") as s_cast,
        nc.semaphore("s_out") as s_out,
        nc.Block() as block,
    ):
        s_a2 = [s_a2_0, s_a2_1, s_a2_2, s_a2_3, s_a2_4]
        s_w2 = [s_w2_0, s_w2_1, s_w2_2, s_w2_3, s_w2_4]

        @block.sync
        def _(sync):
            # input DMAs in consumption order; one semaphore per DMA so a
            # full-value wait is race-free (HWDGE completions can skew).
            c0, cs = CHUNKS[0]
            sync.dma_start(a2_sb[:, 0, :], a2[:, 0, :]).then_inc(s_a2[0], 16)
            sync.dma_start(
                w2_sb[:, :, c0 : c0 + cs], w2[:, :, c0 : c0 + cs]
            ).then_inc(s_w2[0], 16)
            for kc in range(1, KC):
                sync.dma_start(a2_sb[:, kc, :], a2[:, kc, :]).then_inc(s_a2[kc], 16)
            for ci in range(1, len(CHUNKS)):
                c0, cs = CHUNKS[ci]
                sync.dma_start(
                    w2_sb[:, :, c0 : c0 + cs], w2[:, :, c0 : c0 + cs]
                ).then_inc(s_w2[ci], 16)
            # output DMAs, one per (chunk, group), as casts complete
            for i, (ci, g) in enumerate(groups):
                c0, cs = CHUNKS[ci]
                sync.wait_ge(s_cast, i + 1)
                sync.dma_start(
                    out_r[:, g : g + 1, c0 : c0 + cs], ot_sb[:, g : g + 1, c0 : c0 + cs]
                ).then_inc(s_out, 16)
            sync.wait_ge(s_out, 16 * len(groups))

        @block.tensor
        def _(tensor):
            for i, (ci, g) in enumerate(groups):
                c0, cs = CHUNKS[ci]
                bank = i % NBANK
                if i >= NBANK:
                    tensor.wait_ge(s_cast, i - NBANK + 1)
                if g == 0:
                    tensor.wait_ge(s_w2[ci], 16)
                for kc in range(KC):
                    if ci == 0 and g == 0:
                        tensor.wait_ge(s_a2[kc], 16)
                    mm = nc.tensor.matmul(
                        ps[:, bank, :cs],
                        lhsT=a2_sb[:, kc, g * 128 : (g + 1) * 128],
                        rhs=w2_sb[:, kc, c0 : c0 + cs],
                        start=(kc == 0),
                        stop=(kc == KC - 1),
                    )
                    if kc == KC - 1:
                        mm.then_inc(s_mm, 1)

        @block.vector
        def _(vector):
            for i, (ci, g) in enumerate(groups):
                c0, cs = CHUNKS[ci]
                bank = i % NBANK
                vector.wait_ge(s_mm, i + 1)
                nc.vector.tensor_copy(
                    ot_sb[:, g, c0 : c0 + cs], ps[:, bank, :cs]
                ).then_inc(s_cast, 1)

    nc.finalize()
    _cache["nc_raw"] = nc
    return nc


def _build_program():
    if "nc" in _cache:
        return _cache["nc"]
    nc = bacc.Bacc(None)
    f16 = mybir.dt.float16
    f32 = mybir.dt.float32
    w2 = nc.declare_dram_parameter("w2", [128, KC, VSH], f16, isOutput=False)
    a2 = nc.declare_dram_parameter("a2", [128, KC, BM], f16, isOutput=False)
    out = nc.declare_dram_parameter("out", [NG, 128, VSH], f16, isOutput=True)
    out_r = out.rearrange("g p v -> p g v")

    with TileContext(nc) as tc:
        with (
            tc.tile_pool(name="a2p", bufs=1) as a2p,
            tc.tile_pool(name="w2p", bufs=3) as w2p,
            tc.tile_pool(name="op", bufs=2) as op,
            tc.tile_pool(name="psp", bufs=6, space="PSUM") as psp,
        ):
            # per-kc A2 tiles so the first matmul only waits on 1/5 of A2
            a2ts = []
            for kc in range(KC):
                t = a2p.tile([128, BM], f16, tag=f"a2_{kc}")
                a2ts.append(t)
            first = True
            for c0, cs in CHUNKS:
                w2t = w2p.tile([128, KC, cs], f16, tag="w2")
                if first:
                    nc.sync.dma_start(a2ts[0][:], a2[:, 0, :])
                nc.sync.dma_start(w2t[:], w2[:, :, c0 : c0 + cs])
                if first:
                    for kc in range(1, KC):
                        nc.sync.dma_start(a2ts[kc][:], a2[:, kc, :])
                    first = False
                ot = op.tile([128, NG, cs], f16, tag="ot")
                for g in range(NG):
                    ps = psp.tile([128, cs], f32, tag="ps")
                    for kc in range(KC):
                        nc.tensor.matmul(
                            ps[:],
                            lhsT=a2ts[kc][:, g * 128 : (g + 1) * 128],
                            rhs=w2t[:, kc, :],
                            start=(kc == 0),
                            stop=(kc == KC - 1),
                        )
                    nc.vector.tensor_copy(ot[:, g, :], ps[:])
                nc.sync.dma_start(out_r[:, :, c0 : c0 + cs], ot[:])
    nc.finalize()
    _cache["nc"] = nc
    return nc


def _rodrigues(r):
    theta = np.linalg.norm(r, axis=-1, keepdims=True) + 1e-8
    rh = r / theta
    c = np.cos(theta)[..., None]
    s = np.sin(theta)[..., None]
    x, y, z = rh[..., 0], rh[..., 1], rh[..., 2]
    zero = np.zeros_like(x)
    K = np.stack([zero, -z, y, z, zero, -x, -y, x, zero], axis=-1).reshape(
        r.shape[:-1] + (3, 3)
    )
    eye = np.eye(3, dtype=r.dtype)
    outer = rh[..., :, None] * rh[..., None, :]
    return c * eye + (1.0 - c) * outer + s * K


def kernel(
    pose_rot_vec,
    pose_trans,
    v_template,
    weights,
    tpose_joints,
    parents,
    kp_v_ids,
    kp_j_ids,
    kp_is_joint,
):
    pose_rot_vec = np.asarray(pose_rot_vec, dtype=np.float32)
    pose_trans = np.asarray(pose_trans, dtype=np.float32)
    v_template = np.asarray(v_template, dtype=np.float32)
    weights = np.asarray(weights, dtype=np.float32)
    tpose_joints = np.asarray(tpose_joints, dtype=np.float32)
    parents_np = np.asarray(parents)
    kp_v_ids = np.asarray(kp_v_ids)
    kp_j_ids = np.asarray(kp_j_ids)
    kp_is_joint = np.asarray(kp_is_joint)

    b = pose_rot_vec.shape[0]
    # ---- host: Rodrigues + FK along the tree (tiny serial compute) ----
    R = _rodrigues(pose_rot_vec)  # [B,J,3,3]
    S = np.zeros((b, J, 4, 4), dtype=np.float32)
    S[:, :, :3, :3] = R
    S[:, :, :3, 3] = pose_trans
    S[:, :, 3, 3] = 1.0
    G = np.empty_like(S)
    G[:, 0] = S[:, 0]
    for j in range(1, J):
        G[:, j] = G[:, int(parents_np[j])] @ S[:, j]
    J_final = np.ascontiguousarray(G[:, :, :3, 3])  # [B,J,3]
    A = G.copy()
    A[:, :, :3, 3] -= np.einsum("bjmn,jn->bjm", G[:, :, :3, :3], tpose_joints)

    # ---- host: pack device operands ----
    # A2[(j,n),(b,m)] = A[b,j,m,n]; W2[(j,n), v] = w[v,j]*vh[v,n]
    A2 = A[:, :, :3, :].transpose(1, 3, 0, 2).reshape(J * 4, b * 3)
    vh = np.concatenate([v_template, np.ones((VN, 1), np.float32)], axis=1)  # [VN,4]
    W2 = (weights[:, :, None] * vh[:, None, :]).reshape(VN, J * 4)

    kdim = (KCD if USE_FP8 else KC) * 128
    dt_in = mybir.dt.np(mybir.dt.float8e4) if USE_FP8 else np.float16
    kc = KCD if USE_FP8 else KC
    vshp = VSHP if USE_FP8 else VSH

    A2p = np.zeros((kdim, BM), dtype=dt_in)
    A2p[: J * 4] = A2.astype(dt_in)
    A2h = np.ascontiguousarray(A2p.reshape(kc, 128, BM).transpose(1, 0, 2))

    W2p = np.zeros((kdim, VN), dtype=dt_in)
    W2p[: J * 4] = W2.astype(dt_in).T
    W2h = np.ascontiguousarray(W2p.reshape(kc, 128, VN).transpose(1, 0, 2))

    nc = _build_program_fp8() if USE_FP8 else _build_program()
    in_maps = []
    for i in range(NCORES):
        w2c = np.zeros((128, kc, vshp), dtype=dt_in)
        w2c[:, :, :VSH] = W2h[:, :, i * VSH : (i + 1) * VSH]
        in_maps.append({"w2": w2c, "a2": A2h})
    res = run_bass_kernel_spmd(nc, in_maps, core_ids=list(range(NCORES)))
    kernel._last = res

    outs = [
        res.results[i]["out"].reshape(BM, vshp)[:, :VSH] for i in range(NCORES)
    ]  # each [BM, VSH] f16
    Vt = np.concatenate(outs, axis=1)  # [768, 14000]
    V_final = np.ascontiguousarray(
        Vt.reshape(b, 3, VN).transpose(0, 2, 1).astype(np.float32)
    )

    # ---- host: keypoints (tiny gathers) ----
    kp_v = V_final[:, kp_v_ids].mean(axis=2)  # [B,22,3]
    kp_j = J_final[:, kp_j_ids].mean(axis=2)  # [B,22,3]
    keypoints = np.where(kp_is_joint[None, :, None], kp_j, kp_v).astype(np.float32)
    return V_final, J_final, keypoints


# revision 32
# speedup vs baseline: 1.1954x; 1.1173x over previous
"""Articulation (LBS) kernel for 8 TRN2 NeuronCores.

Math: V[b,v,m] = sum_j w[v,j] * (A[b,j,m,:] . vh[v,:]), vh = [v_template, 1].
Fused as a single matmul with K = J*4 = 560 (padded 640):
  out[(b,m), v] = sum_k A2[k, (b,m)] * W2[k, v]
where A2[(j,n),(b,m)] = A[b,j,m,n] (stationary, tiny, replicated) and
W2[(j,n), v] = w[v,j]*vh[v,n] (moving, large, sharded over v across cores).

Host does the tiny serial work (Rodrigues, FK tree walk, keypoints); the
device does the O(B*VN*J) blend. VN=14000 is split 1750/core across 8 cores.
"""

import sys

sys.path.insert(0, "/opt/trn_rl_repo")

import numpy as np

import concourse.bacc as bacc
import concourse.bass as bass
from concourse import mybir
from concourse.bass_utils import run_bass_kernel_spmd
from concourse.tile import TileContext

B, J, VN = 256, 140, 14000
NCORES = 8
VSH = VN // NCORES          # 1750 vertices per core
KC = 5                      # k chunks of 128 (J*4 = 560 padded to 640)
BM = B * 3                  # 768 output rows (b, m)
NG = BM // 128              # 6 stationary groups of 128 output columns
_SIZES = [512, 512, 512, 214]  # big chunks: each compute window covers next DMA
CHUNKS = []
_o = 0
for _s in _SIZES:
    CHUNKS.append((_o, _s))
    _o += _s
assert _o == VSH

_cache = {}

USE_FP8 = True
KCD = 5                      # fp8: 2 DoubleRow pairs (K 0..511) + 1 plain MM (48-row tail)
VSHP = 1760                  # per-core v columns padded to %16 for DoubleRow APs
CHUNKS8 = [(0, 512), (512, 512), (1024, 512), (1536, 224)]


def _build_program_fp8():
    """fp8 e4m3 + DoubleRow: 2 K-rows per PE cell -> 3 passes instead of 5.
    Casts split 2:1 between DVE and ACT (both may read PSUM, different banks)."""
    if "nc8" in _cache:
        return _cache["nc8"]
    nc = bacc.Bacc(None)
    f8 = mybir.dt.float8e4
    f16 = mybir.dt.float16
    f32 = mybir.dt.float32
    w2 = nc.declare_dram_parameter("w2", [128, KCD, VSHP], f8, isOutput=False)
    a2 = nc.declare_dram_parameter("a2", [128, KCD, BM], f8, isOutput=False)
    out = nc.declare_dram_parameter("out", [NG, 128, VSHP], f16, isOutput=True)
    out_r = out.rearrange("g p v -> p g v")

    with TileContext(nc) as tc:
        with (
            tc.tile_pool(name="a2p", bufs=1) as a2p,
            tc.tile_pool(name="w2p", bufs=3) as w2p,
            tc.tile_pool(name="op", bufs=2) as op,
            tc.tile_pool(name="psp", bufs=7, space="PSUM") as psp,
            tc.tile_pool(name="wpsp", bufs=1, space="PSUM") as wpsp,
        ):
            a2t0 = a2p.tile([128, 2, BM], f8, tag="a2_0")
            a2t1 = a2p.tile([128, 2, BM], f8, tag="a2_1")
            a2ts = [a2t0, a2t1]
            a2tail = a2p.tile([128, BM], f8, tag="a2_t")
            # HAM warmup: keep PE busy during the input-DMA lead so the real
            # stream starts at 2.4 GHz.
            wrm = a2p.tile([128, 512], f16, tag="wrm")
            nc.gpsimd.memset(wrm[:], 0.0)
            wps = wpsp.tile([128, 512], f32, tag="wps")
            for _ in range(10):
                nc.tensor.matmul(
                    wps[:], lhsT=wrm[:, :128], rhs=wrm[:], start=True, stop=True
                )
            first = True
            gi = 0
            for c0, cs in CHUNKS8:
                w2t = w2p.tile([128, KCD, cs], f8, tag="w2")
                if first:
                    nc.sync.dma_start(a2ts[0][:], a2[:, 0:2, :])
                nc.sync.dma_start(w2t[:], w2[:, :, c0 : c0 + cs])
                if first:
                    nc.sync.dma_start(a2ts[1][:], a2[:, 2:4, :])
                    nc.sync.dma_start(a2tail[:], a2[:, 4, :])
                    first = False
                ot = op.tile([128, NG, cs], f16, tag="ot")
                for g in range(NG):
                    ps = psp.tile([128, cs], f32, tag="ps")
                    for p in range(2):
                        nc.tensor.matmul(
                            ps[:],
                            lhsT=a2ts[p][:, :, g * 128 : (g + 1) * 128],
                            rhs=w2t[:, 2 * p : 2 * p + 2, :],
                            start=(p == 0),
                            stop=False,
                            perf_mode=mybir.MatmulPerfMode.DoubleRow,
                        )
                    nc.tensor.matmul(
                        ps[:],
                        lhsT=a2tail[:, g * 128 : (g + 1) * 128],
                        rhs=w2t[:, 4, :],
                        start=False,
                        stop=True,
                    )
                    if gi % 3 == 2:
                        nc.scalar.copy(ot[:, g, :], ps[:])
                    else:
                        nc.vector.tensor_copy(ot[:, g, :], ps[:])
                    gi += 1
                nc.sync.dma_start(out_r[:, :, c0 : c0 + cs], ot[:])
    nc.finalize()
    _cache["nc8"] = nc
    return nc


def _build_program_raw():
    """Hand-scheduled raw-Bass version: explicit engine streams + 4 semaphores.

    Avoids Tile/Bacc's event-semaphore system whose prologue/teardown
    (all-engine barriers, ~253 per-sem resets) costs ~15us of fixed time.
    """
    if "nc_raw" in _cache:
        return _cache["nc_raw"]
    nc = bass.Bass()
    f16 = mybir.dt.float16
    f32 = mybir.dt.float32
    w2 = nc.declare_dram_parameter("w2", [128, KC, VSH], f16, isOutput=False)
    a2 = nc.declare_dram_parameter("a2", [128, KC, BM], f16, isOutput=False)
    out = nc.declare_dram_parameter("out", [NG, 128, VSH], f16, isOutput=True)
    out_r = out.rearrange("g p v -> p g v")

    NBANK = 8
    groups = [(ci, g) for ci in range(len(CHUNKS)) for g in range(NG)]

    with (
        nc.sbuf_tensor([128, KC, BM], f16) as a2_sb,
        nc.sbuf_tensor([128, KC, VSH], f16) as w2_sb,
        nc.sbuf_tensor([128, NG, VSH], f16) as ot_sb,
        nc.psum_tensor([128, NBANK, 512], f32) as ps,
        nc.semaphore("s_a2_0") as s_a2_0,
        nc.semaphore("s_a2_1") as s_a2_1,
        nc.semaphore("s_a2_2") as s_a2_2,
        nc.semaphore("s_a2_3") as s_a2_3,
        nc.semaphore("s_a2_4") as s_a2_4,
        nc.semaphore("s_w2_0") as s_w2_0,
        nc.semaphore("s_w2_1") as s_w2_1,
        nc.semaphore("s_w2_2") as s_w2_2,
        nc.semaphore("s_w2_3") as s_w2_3,
        nc.semaphore("s_w2_4") as s_w2_4,
        nc.semaphore("s_mm") as s_mm,
        nc.semaphore("s_cast") as s_cast,
        nc.semaphore("s_out") as s_out,
        nc.Block() as block,
    ):
        s_a2 = [s_a2_0, s_a2_1, s_a2_2, s_a2_3, s_a2_4]
        s_w2 = [s_w2_0, s_w2_1, s_w2_2, s_w2_3, s_w2_4]

        @block.sync
        def _(sync):
            # input DMAs in consumption order; one semaphore per DMA so a
            # full-value wait is race-free (HWDGE completions can skew).
            c0, cs = CHUNKS[0]
            sync.dma_start(a2_sb[:, 0, :], a2[:, 0, :]).then_inc(s_a2[0], 16)
            sync.dma_start(
                w2_sb[:, :, c0 : c0 + cs], w2[:, :, c0 : c0 + cs]
            ).then_inc(s_w2[0], 16)
            for kc in range(1, KC):
                sync.dma_start(a2_sb[:, kc, :], a2[:, kc, :]).then_inc(s_a2[kc], 16)
            for ci in range(1, len(CHUNKS)):
                c0, cs = CHUNKS[ci]
                sync.dma_start(
                    w2_sb[:, :, c0 : c0 + cs], w2[:, :, c0 : c0 + cs]
                ).then_inc(s_w2[ci], 16)
            # output DMAs, one per (chunk, group), as casts complete
            for i, (ci, g) in enumerate(groups):
                c0, cs = CHUNKS[ci]
                sync.wait_ge(s_cast, i + 1)
                sync.dma_start(
                    out_r[:, g : g + 1, c0 : c0 + cs], ot_sb[:, g : g + 1, c0 : c0 + cs]
                ).then_inc(s_out, 16)
            sync.wait_ge(s_out, 16 * len(groups))

        @block.tensor
        def _(tensor):
            for i, (ci, g) in enumerate(groups):
                c0, cs = CHUNKS[ci]
                bank = i % NBANK
                if i >= NBANK:
                    tensor.wait_ge(s_cast, i - NBANK + 1)
                if g == 0:
                    tensor.wait_ge(s_w2[ci], 16)
                for kc in range(KC):
                    if ci == 0 and g == 0:
                        tensor.wait_ge(s_a2[kc], 16)
                    mm = nc.tensor.matmul(
                        ps[:, bank, :cs],
                        lhsT=a2_sb[:, kc, g * 128 : (g + 1) * 128],
                        rhs=w2_sb[:, kc, c0 : c0 + cs],
                        start=(kc == 0),
                        stop=(kc == KC - 1),
                    )
                    if kc == KC - 1:
                        mm.then_inc(s_mm, 1)

        @block.vector
        def _(vector):
            for i, (ci, g) in enumerate(groups):
                c0, cs = CHUNKS[ci]
                bank = i % NBANK
                vector.wait_ge(s_mm, i + 1)
                nc.vector.tensor_copy(
                    ot_sb[:, g, c0 : c0 + cs], ps[:, bank, :cs]
                ).then_inc(s_cast, 1)

    nc.finalize()
    _cache["nc_raw"] = nc
    return nc


def _build_program():
    if "nc" in _cache:
        return _cache["nc"]
    nc = bacc.Bacc(None)
    f16 = mybir.dt.float16
    f32 = mybir.dt.float32
    w2 = nc.declare_dram_parameter("w2", [128, KC, VSH], f16, isOutput=False)
    a2 = nc.declare_dram_parameter("a2", [128, KC, BM], f16, isOutput=False)
    out = nc.declare_dram_parameter("out", [NG, 128, VSH], f16, isOutput=True)
    out_r = out.rearrange("g p v -> p g v")

    with TileContext(nc) as tc:
        with (
            tc.tile_pool(name="a2p", bufs=1) as a2p,
            tc.tile_pool(name="w2p", bufs=3) as w2p,
            tc.tile_pool(name="op", bufs=2) as op,
            tc.tile_pool(name="psp", bufs=6, space="PSUM") as psp,
        ):
            # per-kc A2 tiles so the first matmul only waits on 1/5 of A2
            a2ts = []
            for kc in range(KC):
                t = a2p.tile([128, BM], f16, tag=f"a2_{kc}")
                a2ts.append(t)
            first = True
            for c0, cs in CHUNKS:
                w2t = w2p.tile([128, KC, cs], f16, tag="w2")
                if first:
                    nc.sync.dma_start(a2ts[0][:], a2[:, 0, :])
                nc.sync.dma_start(w2t[:], w2[:, :, c0 : c0 + cs])
                if first:
                    for kc in range(1, KC):
                        nc.sync.dma_start(a2ts[kc][:], a2[:, kc, :])
                    first = False
                ot = op.tile([128, NG, cs], f16, tag="ot")
                for g in range(NG):
                    ps = psp.tile([128, cs], f32, tag="ps")
                    for kc in range(KC):
                        nc.tensor.matmul(
                            ps[:],
                            lhsT=a2ts[kc][:, g * 128 : (g + 1) * 128],
                            rhs=w2t[:, kc, :],
                            start=(kc == 0),
                            stop=(kc == KC - 1),
                        )
                    nc.vector.tensor_copy(ot[:, g, :], ps[:])
                nc.sync.dma_start(out_r[:, :, c0 : c0 + cs], ot[:])
    nc.finalize()
    _cache["nc"] = nc
    return nc


def _rodrigues(r):
    theta = np.linalg.norm(r, axis=-1, keepdims=True) + 1e-8
    rh = r / theta
    c = np.cos(theta)[..., None]
    s = np.sin(theta)[..., None]
    x, y, z = rh[..., 0], rh[..., 1], rh[..., 2]
    zero = np.zeros_like(x)
    K = np.stack([zero, -z, y, z, zero, -x, -y, x, zero], axis=-1).reshape(
        r.shape[:-1] + (3, 3)
    )
    eye = np.eye(3, dtype=r.dtype)
    outer = rh[..., :, None] * rh[..., None, :]
    return c * eye + (1.0 - c) * outer + s * K


def kernel(
    pose_rot_vec,
    pose_trans,
    v_template,
    weights,
    tpose_joints,
    parents,
    kp_v_ids,
    kp_j_ids,
    kp_is_joint,
):
    pose_rot_vec = np.asarray(pose_rot_vec, dtype=np.float32)
    pose_trans = np.asarray(pose_trans, dtype=np.float32)
    v_template = np.asarray(v_template, dtype=np.float32)
    weights = np.asarray(weights, dtype=np.float32)
    tpose_joints = np.asarray(tpose_joints, dtype=np.float32)
    parents_np = np.asarray(parents)
    kp_v_ids = np.asarray(kp_v_ids)
    kp_j_ids = np.asarray(kp_j_ids)
    kp_is_joint = np.asarray(kp_is_joint)

    b = pose_rot_vec.shape[0]
    # ---- host: Rodrigues + FK along the tree (tiny serial compute) ----
    R = _rodrigues(pose_rot_vec)  # [B,J,3,3]
    S = np.zeros((b, J, 4, 4), dtype=np.float32)
    S[:, :, :3, :3] = R
    S[:, :, :3, 3] = pose_trans
    S[:, :, 3, 3] = 1.0
    G = np.empty_like(S)
    G[:, 0] = S[:, 0]
    for j in range(1, J):
        G[:, j] = G[:, int(parents_np[j])] @ S[:, j]
    J_final = np.ascontiguousarray(G[:, :, :3, 3])  # [B,J,3]
    A = G.copy()
    A[:, :, :3, 3] -= np.einsum("bjmn,jn->bjm", G[:, :, :3, :3], tpose_joints)

    # ---- host: pack device operands ----
    # A2[(j,n),(b,m)] = A[b,j,m,n]; W2[(j,n), v] = w[v,j]*vh[v,n]
    A2 = A[:, :, :3, :].transpose(1, 3, 0, 2).reshape(J * 4, b * 3)
    vh = np.concatenate([v_template, np.ones((VN, 1), np.float32)], axis=1)  # [VN,4]
    W2 = (weights[:, :, None] * vh[:, None, :]).reshape(VN, J * 4)

    kdim = (KCD if USE_FP8 else KC) * 128
    dt_in = mybir.dt.np(mybir.dt.float8e4) if USE_FP8 else np.float16
    kc = KCD if USE_FP8 else KC
    vshp = VSHP if USE_FP8 else VSH

    A2p = np.zeros((kdim, BM), dtype=dt_in)
    A2p[: J * 4] = A2.astype(dt_in)
    A2h = np.ascontiguousarray(A2p.reshape(kc, 128, BM).transpose(1, 0, 2))

    W2p = np.zeros((kdim, VN), dtype=dt_in)
    W2p[: J * 4] = W2.astype(dt_in).T
    W2h = np.ascontiguousarray(W2p.reshape(kc, 128, VN).transpose(1, 0, 2))

    nc = _build_program_fp8() if USE_FP8 else _build_program()
    in_maps = []
    for i in range(NCORES):
        w2c = np.zeros((128, kc, vshp), dtype=dt_in)
        w2c[:, :, :VSH] = W2h[:, :, i * VSH : (i + 1) * VSH]
        in_maps.append({"w2": w2c, "a2": A2h})
    res = run_bass_kernel_spmd(nc, in_maps, core_ids=list(range(NCORES)))
    kernel._last = res

    outs = [
        res.results[i]["out"].reshape(BM, vshp)[:, :VSH] for i in range(NCORES)
    ]  # each [BM, VSH] f16
    Vt = np.concatenate(outs, axis=1)  # [768, 14000]
    V_final = np.ascontiguousarray(
        Vt.reshape(b, 3, VN).transpose(0, 2, 1).astype(np.float32)
    )

    # ---- host: keypoints (tiny gathers) ----
    kp_v = V_final[:, kp_v_ids].mean(axis=2)  # [B,22,3]
    kp_j = J_final[:, kp_j_ids].mean(axis=2)  # [B,22,3]
    keypoints = np.where(kp_is_joint[None, :, None], kp_j, kp_v).astype(np.float32)
    return V_final, J_final, keypoints
